# revision 16
# baseline (speedup 1.0000x reference)
"""Trainium2 Bass kernel: equivariant block-diagonal linear (irreps 0e/1o/2e).

y[n, base_d + v*d + i] = (1/sqrt(256)) * sum_u W_d[u, v] * x[n, base_d + u*d + i]

Strategy (data-parallel over 8 NeuronCores, 4096 nodes per core):
  - per 128-node chunk: contiguous DMA of x [128, 2304] into SBUF
  - PE transpose (matmul with identity) of 18 [128,128] feature blocks,
    using stride-d feature APs to de-interleave the irrep components
  - matmuls with xT as stationary operand, weights as moving operand
    (float32r for full-rate PE), accumulating over the two 128-row u-chunks
  - PSUM -> SBUF copyback with stride-d writes rebuilds the mul_ir layout
  - contiguous DMA of y [128, 2304] back to DRAM
Weights are pre-scaled by 1/16 and rearranged to [128, 1536] on the host.
"""

import sys

if "/opt/trn_rl_repo" not in sys.path:
    sys.path.insert(0, "/opt/trn_rl_repo")

from contextlib import ExitStack

import numpy as np

import concourse.bass as bass
import concourse.mybir as mybir
import concourse.tile as tile
from concourse.bass_utils import run_bass_kernel_spmd
from concourse.masks import make_identity

P = 128
N_CORES = 8
N_NODES = 32768
IN_DIM = 2304
IRREPS = [(256, 1), (256, 3), (256, 5)]
BASES = [0, 256, 1024]  # feature offset of each irrep block
N_PER_CORE = N_NODES // N_CORES  # 4096
N_CHUNKS = N_PER_CORE // P  # 32
USE_F32R = True

# (irrep, i, u_chunk) triples in fixed order; groups of 4 share a PSUM bank
TRIPLES = [
    (ir, i, uc)
    for ir, (_, d) in enumerate(IRREPS)
    for i in range(d)
    for uc in range(2)
]
TRIPLE_IDX = {t: k for k, t in enumerate(TRIPLES)}
N_GROUPS = (len(TRIPLES) + 3) // 4  # 5 (4+4+4+4+2)


def _build(n_chunks: int, f32r: bool, split_waits: bool = True) -> bass.Bass:
    f32 = mybir.dt.float32
    # float32r = same bits as fp32, full-rate PE matmul (vs 4 cycles/row for
    # fp32). The BIR verifier requires fp32r matmul operands to be *produced*
    # as fp32r, so the weight tensors are declared fp32r end-to-end and the
    # xT copyback casts fp32 -> fp32r.
    mm_dt = mybir.dt.float32r if f32r else f32
    nc = bass.Bass("TRN2", target_bir_lowering=False, debug=False)
    x = nc.dram_tensor("x", [n_chunks * P, IN_DIM], f32, kind="ExternalInput").ap()
    w = nc.dram_tensor("w", [P, 1536], mm_dt, kind="ExternalInput").ap()
    y = nc.dram_tensor("y", [n_chunks * P, IN_DIM], f32, kind="ExternalOutput").ap()

    # batch DMAs over CG chunks: one [CG*128, 2304] transfer amortizes the
    # per-DMA ramp (1.18 MB -> ~78% of peak; 4.7 MB -> ~90%)
    CG = 4
    assert n_chunks % CG == 0

    with tile.TileContext(nc) as tc, ExitStack() as ctx:
        const_pool = ctx.enter_context(tc.tile_pool(name="const", bufs=1))
        x_pool = ctx.enter_context(tc.tile_pool(name="x", bufs=2))
        y_pool = ctx.enter_context(tc.tile_pool(name="y", bufs=2))
        xt_pool = ctx.enter_context(tc.tile_pool(name="xt", bufs=6))
        tpsum_pool = ctx.enter_context(tc.tile_pool(name="tpsum", bufs=2, space="PSUM"))
        ypsum_pool = ctx.enter_context(tc.tile_pool(name="ypsum", bufs=3, space="PSUM"))

        w_tile = const_pool.tile([P, 1536], mm_dt)
        nc.sync.dma_start(w_tile[:], w[:, :])
        ident = const_pool.tile([P, P], f32)
        make_identity(nc, ident[:])

        # Dummy PE ops absorb the one-time identity (Pool sem) and weight-DMA
        # waits so the first real matmuls start with a single wait.
        dummy_pool = ctx.enter_context(tc.tile_pool(name="dummy", bufs=1, space="PSUM"))
        scratch = dummy_pool.tile([P, 256], f32)
        nc.tensor.transpose(scratch[:, :P], ident[:], ident[:])
        nc.tensor.matmul(
            scratch[:, :256], w_tile[:, :P], w_tile[:, :256], start=True, stop=True
        )

        # component pairs per irrep for the y copyback (two 256-wide PSUM
        # halves share a bank and copy out in one strided op)
        Y_PAIRS = [
            (ir, i0, min(2, d - i0))
            for ir, (_, d) in enumerate(IRREPS)
            for i0 in range(0, d, 2)
        ]

        n_cg = n_chunks // CG
        for cg in range(n_cg):
            xg = x_pool.tile([P, CG, IN_DIM], f32)
            # first/last group: per-chunk transfers so compute starts sooner
            # and the drain tail is fine-grained; middle groups: one big
            # transfer at peak DMA efficiency
            x_view = x[cg * CG * P : (cg + 1) * CG * P, :].rearrange(
                "(g p) f -> p g f", p=P
            )
            if cg in (0, n_cg - 1):
                for g in range(CG):
                    nc.sync.dma_start(xg[:, g : g + 1, :], x_view[:, g : g + 1, :])
            else:
                nc.sync.dma_start(xg[:], x_view)
            yg = y_pool.tile([P, CG, IN_DIM], f32)

            for g in range(CG):
                x_t = xg[:, g, :]
                # transpose 18 [128,128] blocks of x, 4 per PSUM bank;
                # copyback on DVE
                xt_tiles = []
                for tg in range(N_GROUPS):
                    group = TRIPLES[tg * 4 : (tg + 1) * 4]
                    ps = tpsum_pool.tile([P, 512], f32)
                    xt = xt_pool.tile([P, 512], mm_dt)
                    for t, (ir, i, uc) in enumerate(group):
                        d = IRREPS[ir][1]
                        start = BASES[ir] + uc * P * d + i
                        nc.tensor.transpose(
                            ps[:, t * P : (t + 1) * P],
                            x_t[:, start : start + (P - 1) * d + 1 : d],
                            ident[:],
                        )
                    width = len(group) * P
                    nc.vector.tensor_copy(xt[:, :width], ps[:, :width])
                    xt_tiles.append(xt)

                # block matmuls: out[n, v] += xT[u, n].T @ W[u, v]; two
                # components accumulate into one PSUM bank, then one strided
                # ACT copy rebuilds the mul_ir interleave in SBUF
                for ir, i0, npair in Y_PAIRS:
                    d = IRREPS[ir][1]
                    base = BASES[ir]
                    yp = ypsum_pool.tile([P, 512], f32)
                    for k in range(npair):
                        i = i0 + k
                        for uc in range(2):
                            tg, t = divmod(TRIPLE_IDX[(ir, i, uc)], 4)
                            lhsT = xt_tiles[tg][:, t * P : (t + 1) * P]
                            rhs = w_tile[
                                :, (ir * 2 + uc) * 256 : (ir * 2 + uc + 1) * 256
                            ]
                            nc.tensor.matmul(
                                yp[:, k * 256 : (k + 1) * 256],
                                lhsT,
                                rhs,
                                start=(uc == 0),
                                stop=(uc == 1),
                            )
                    y_view = yg[:, g, base : base + 256 * d].rearrange(
                        "p (v i) -> p i v", i=d
                    )
                    nc.scalar.copy(
                        y_view[:, i0 : i0 + npair, :],
                        yp[:, : npair * 256].rearrange("p (i v) -> p i v", v=256),
                    )

            y_view = y[cg * CG * P : (cg + 1) * CG * P, :].rearrange(
                "(g p) f -> p g f", p=P
            )
            if cg == n_cg - 1:
                for g in range(CG):
                    nc.scalar.dma_start(y_view[:, g : g + 1, :], yg[:, g : g + 1, :])
            else:
                # two half-group transfers so the first half streams out while
                # the second half still computes
                nc.scalar.dma_start(y_view[:, 0:2, :], yg[:, 0:2, :])
                nc.scalar.dma_start(y_view[:, 2:4, :], yg[:, 2:4, :])

    if split_waits:
        # CoreSim's race detector rejects hand-inserted instructions, so this
        # only runs for hardware builds; it does not change semantics.
        _split_matmul_waits(nc)
    return nc


def _split_matmul_waits(nc: bass.Bass) -> None:
    """Walrus codegen supports only one semaphore wait per instruction (two on
    InstEventSemaphore). Move excess waits onto standalone InstEventSemaphore
    instructions inserted just before, on the same engine queue — semantically
    identical, the engine blocks on all of them either way."""

    def fix_block(block):
        new = []
        for inst in block.instructions:
            si = getattr(inst, "sync_info", None)
            cap = 2 if isinstance(inst, mybir.InstEventSemaphore) else 1
            if si is not None and si.on_wait and len(si.on_wait) > cap:
                waits = list(si.on_wait)
                move, keep = waits[:-cap], waits[-cap:]
                for j in range(0, len(move), 2):
                    new.append(
                        mybir.InstEventSemaphore(
                            name=f"{inst.name}-prewait{j}",
                            engine=inst.engine,
                            ins=[],
                            outs=[],
                            sync_info=mybir.SyncInfo(
                                on_wait=move[j : j + 2], on_update=[]
                            ),
                        )
                    )
                si.on_wait = keep
            new.append(inst)
        block.instructions = new
        for b in getattr(block, "blocks", []):
            fix_block(b)

    for f in nc.m.functions:
        for b in f.blocks:
            fix_block(b)


_NC_CACHE: dict = {}


def _get_nc(n_chunks: int, f32r: bool) -> bass.Bass:
    key = (n_chunks, f32r)
    if key not in _NC_CACHE:
        _NC_CACHE[key] = _build(n_chunks, f32r)
    return _NC_CACHE[key]


def _arrange_weights(weights: np.ndarray) -> np.ndarray:
    """[196608] flat -> [128, 1536]: per irrep, the two 128-row u-chunks of
    (W / sqrt(mul)) side by side as [128, 256] blocks."""
    w = np.asarray(weights, dtype=np.float32)
    blocks = []
    wo = 0
    for mul, _ in IRREPS:
        W = w[wo : wo + mul * mul].reshape(mul, mul) * np.float32(
            1.0 / np.sqrt(np.float32(mul))
        )
        blocks.append(W[:P, :])
        blocks.append(W[P:, :])
        wo += mul * mul
    return np.ascontiguousarray(np.concatenate(blocks, axis=1), dtype=np.float32)


def _run(x: np.ndarray, weights: np.ndarray, trace: bool = False, f32r: bool = USE_F32R):
    x = np.ascontiguousarray(np.asarray(x), dtype=np.float32)
    assert x.shape == (N_NODES, IN_DIM), x.shape
    w_arr = _arrange_weights(weights)
    nc = _get_nc(N_CHUNKS, f32r)
    in_maps = [
        {"x": x[c * N_PER_CORE : (c + 1) * N_PER_CORE], "w": w_arr}
        for c in range(N_CORES)
    ]
    res = run_bass_kernel_spmd(nc, in_maps, list(range(N_CORES)), trace=trace)
    y = np.concatenate([r["y"] for r in res.results], axis=0)
    return y, res


def kernel(x: np.ndarray, weights: np.ndarray) -> np.ndarray:
    y, _ = _run(x, weights)
    return y


# revision 17
# speedup vs baseline: 1.1558x; 1.1558x over previous
"""Trainium2 Bass kernel: equivariant block-diagonal linear (irreps 0e/1o/2e).

y[n, base_d + v*d + i] = (1/sqrt(256)) * sum_u W_d[u, v] * x[n, base_d + u*d + i]

Strategy (data-parallel over 8 NeuronCores, 4096 nodes per core):
  - per 128-node chunk: contiguous DMA of x [128, 2304] into SBUF
  - PE transpose (matmul with identity) of 18 [128,128] feature blocks,
    using stride-d feature APs to de-interleave the irrep components
  - matmuls with xT as stationary operand, weights as moving operand
    (float32r for full-rate PE), accumulating over the two 128-row u-chunks
  - PSUM -> SBUF copyback with stride-d writes rebuilds the mul_ir layout
  - contiguous DMA of y [128, 2304] back to DRAM
Weights are pre-scaled by 1/16 and rearranged to [128, 1536] on the host.
"""

import sys

if "/opt/trn_rl_repo" not in sys.path:
    sys.path.insert(0, "/opt/trn_rl_repo")

from contextlib import ExitStack

import numpy as np

import concourse.bass as bass
import concourse.mybir as mybir
import concourse.tile as tile
from concourse.bass_utils import run_bass_kernel_spmd
from concourse.masks import make_identity

P = 128
N_CORES = 8
N_NODES = 32768
IN_DIM = 2304
IRREPS = [(256, 1), (256, 3), (256, 5)]
BASES = [0, 256, 1024]  # feature offset of each irrep block
N_PER_CORE = N_NODES // N_CORES  # 4096
N_CHUNKS = N_PER_CORE // P  # 32
USE_F32R = True

# (irrep, i, u_chunk) triples in fixed order; groups of 4 share a PSUM bank
TRIPLES = [
    (ir, i, uc)
    for ir, (_, d) in enumerate(IRREPS)
    for i in range(d)
    for uc in range(2)
]
TRIPLE_IDX = {t: k for k, t in enumerate(TRIPLES)}
N_GROUPS = (len(TRIPLES) + 3) // 4  # 5 (4+4+4+4+2)


def _build(n_chunks: int, f32r: bool, split_waits: bool = True) -> bass.Bass:
    f32 = mybir.dt.float32
    # float32r = same bits as fp32, full-rate PE matmul (vs 4 cycles/row for
    # fp32). The BIR verifier requires fp32r matmul operands to be *produced*
    # as fp32r, so the weight tensors are declared fp32r end-to-end and the
    # xT copyback casts fp32 -> fp32r.
    mm_dt = mybir.dt.float32r if f32r else f32
    nc = bass.Bass("TRN2", target_bir_lowering=False, debug=False)
    x = nc.dram_tensor("x", [n_chunks * P, IN_DIM], f32, kind="ExternalInput").ap()
    w = nc.dram_tensor("w", [P, 1536], mm_dt, kind="ExternalInput").ap()
    y = nc.dram_tensor("y", [n_chunks * P, IN_DIM], f32, kind="ExternalOutput").ap()

    # batch DMAs over CG chunks: one [CG*128, 2304] transfer amortizes the
    # per-DMA ramp (1.18 MB -> ~78% of peak; 4.7 MB -> ~90%)
    CG = 4
    assert n_chunks % CG == 0

    with tile.TileContext(nc) as tc, ExitStack() as ctx:
        const_pool = ctx.enter_context(tc.tile_pool(name="const", bufs=1))
        x_pool = ctx.enter_context(tc.tile_pool(name="x", bufs=2))
        y_pool = ctx.enter_context(tc.tile_pool(name="y", bufs=2))
        xt_pool = ctx.enter_context(tc.tile_pool(name="xt", bufs=6))
        tpsum_pool = ctx.enter_context(tc.tile_pool(name="tpsum", bufs=2, space="PSUM"))
        ypsum_pool = ctx.enter_context(tc.tile_pool(name="ypsum", bufs=3, space="PSUM"))

        w_tile = const_pool.tile([P, 1536], mm_dt)
        nc.sync.dma_start(w_tile[:], w[:, :])
        ident = const_pool.tile([P, P], f32)
        make_identity(nc, ident[:])

        # Dummy PE ops absorb the one-time identity (Pool sem) and weight-DMA
        # waits so the first real matmuls start with a single wait.
        dummy_pool = ctx.enter_context(tc.tile_pool(name="dummy", bufs=1, space="PSUM"))
        scratch = dummy_pool.tile([P, 256], f32)
        nc.tensor.transpose(scratch[:, :P], ident[:], ident[:])
        nc.tensor.matmul(
            scratch[:, :256], w_tile[:, :P], w_tile[:, :256], start=True, stop=True
        )

        # component pairs per irrep for the y copyback (two 256-wide PSUM
        # halves share a bank and copy out in one strided op)
        Y_PAIRS = [
            (ir, i0, min(2, d - i0))
            for ir, (_, d) in enumerate(IRREPS)
            for i0 in range(0, d, 2)
        ]

        # group sizes: big groups for DMA efficiency, two small tail groups
        # so the final drain (compute+store after the last x byte) is short
        if n_chunks >= 8:
            group_sizes = [CG] * ((n_chunks - 4) // CG) + [2, 2]
        else:
            group_sizes = [CG] * (n_chunks // CG)
        assert sum(group_sizes) == n_chunks
        c0 = 0
        for gsz in group_sizes:
            xg = x_pool.tile([P, CG, IN_DIM], f32)
            nc.sync.dma_start(
                xg[:, :gsz, :],
                x[c0 * P : (c0 + gsz) * P, :].rearrange("(g p) f -> p g f", p=P),
            )
            yg = y_pool.tile([P, CG, IN_DIM], f32)

            for g in range(gsz):
                x_t = xg[:, g, :]
                # transpose 18 [128,128] blocks of x, 4 per PSUM bank;
                # copyback on DVE
                xt_tiles = []
                for tg in range(N_GROUPS):
                    group = TRIPLES[tg * 4 : (tg + 1) * 4]
                    ps = tpsum_pool.tile([P, 512], f32)
                    xt = xt_pool.tile([P, 512], mm_dt)
                    for t, (ir, i, uc) in enumerate(group):
                        d = IRREPS[ir][1]
                        start = BASES[ir] + uc * P * d + i
                        nc.tensor.transpose(
                            ps[:, t * P : (t + 1) * P],
                            x_t[:, start : start + (P - 1) * d + 1 : d],
                            ident[:],
                        )
                    width = len(group) * P
                    nc.vector.tensor_copy(xt[:, :width], ps[:, :width])
                    xt_tiles.append(xt)

                # block matmuls: out[n, v] += xT[u, n].T @ W[u, v]; two
                # components accumulate into one PSUM bank, then one strided
                # ACT copy rebuilds the mul_ir interleave in SBUF
                for ir, i0, npair in Y_PAIRS:
                    d = IRREPS[ir][1]
                    base = BASES[ir]
                    yp = ypsum_pool.tile([P, 512], f32)
                    for k in range(npair):
                        i = i0 + k
                        for uc in range(2):
                            tg, t = divmod(TRIPLE_IDX[(ir, i, uc)], 4)
                            lhsT = xt_tiles[tg][:, t * P : (t + 1) * P]
                            rhs = w_tile[
                                :, (ir * 2 + uc) * 256 : (ir * 2 + uc + 1) * 256
                            ]
                            nc.tensor.matmul(
                                yp[:, k * 256 : (k + 1) * 256],
                                lhsT,
                                rhs,
                                start=(uc == 0),
                                stop=(uc == 1),
                            )
                    y_view = yg[:, g, base : base + 256 * d].rearrange(
                        "p (v i) -> p i v", i=d
                    )
                    nc.scalar.copy(
                        y_view[:, i0 : i0 + npair, :],
                        yp[:, : npair * 256].rearrange("p (i v) -> p i v", v=256),
                    )

            nc.scalar.dma_start(
                y[c0 * P : (c0 + gsz) * P, :].rearrange("(g p) f -> p g f", p=P),
                yg[:, :gsz, :],
            )
            c0 += gsz

    if split_waits:
        # CoreSim's race detector rejects hand-inserted instructions, so this
        # only runs for hardware builds; it does not change semantics.
        _split_matmul_waits(nc)
    return nc


def _split_matmul_waits(nc: bass.Bass) -> None:
    """Walrus codegen supports only one semaphore wait per instruction (two on
    InstEventSemaphore). Move excess waits onto standalone InstEventSemaphore
    instructions inserted just before, on the same engine queue — semantically
    identical, the engine blocks on all of them either way."""

    def fix_block(block):
        new = []
        for inst in block.instructions:
            si = getattr(inst, "sync_info", None)
            cap = 2 if isinstance(inst, mybir.InstEventSemaphore) else 1
            if si is not None and si.on_wait and len(si.on_wait) > cap:
                waits = list(si.on_wait)
                move, keep = waits[:-cap], waits[-cap:]
                for j in range(0, len(move), 2):
                    new.append(
                        mybir.InstEventSemaphore(
                            name=f"{inst.name}-prewait{j}",
                            engine=inst.engine,
                            ins=[],
                            outs=[],
                            sync_info=mybir.SyncInfo(
                                on_wait=move[j : j + 2], on_update=[]
                            ),
                        )
                    )
                si.on_wait = keep
            new.append(inst)
        block.instructions = new
        for b in getattr(block, "blocks", []):
            fix_block(b)

    for f in nc.m.functions:
        for b in f.blocks:
            fix_block(b)


_NC_CACHE: dict = {}


def _get_nc(n_chunks: int, f32r: bool) -> bass.Bass:
    key = (n_chunks, f32r)
    if key not in _NC_CACHE:
        _NC_CACHE[key] = _build(n_chunks, f32r)
    return _NC_CACHE[key]


def _arrange_weights(weights: np.ndarray) -> np.ndarray:
    """[196608] flat -> [128, 1536]: per irrep, the two 128-row u-chunks of
    (W / sqrt(mul)) side by side as [128, 256] blocks."""
    w = np.asarray(weights, dtype=np.float32)
    blocks = []
    wo = 0
    for mul, _ in IRREPS:
        W = w[wo : wo + mul * mul].reshape(mul, mul) * np.float32(
            1.0 / np.sqrt(np.float32(mul))
        )
        blocks.append(W[:P, :])
        blocks.append(W[P:, :])
        wo += mul * mul
    return np.ascontiguousarray(np.concatenate(blocks, axis=1), dtype=np.float32)


def _run(x: np.ndarray, weights: np.ndarray, trace: bool = False, f32r: bool = USE_F32R):
    x = np.ascontiguousarray(np.asarray(x), dtype=np.float32)
    assert x.shape == (N_NODES, IN_DIM), x.shape
    w_arr = _arrange_weights(weights)
    nc = _get_nc(N_CHUNKS, f32r)
    in_maps = [
        {"x": x[c * N_PER_CORE : (c + 1) * N_PER_CORE], "w": w_arr}
        for c in range(N_CORES)
    ]
    res = run_bass_kernel_spmd(nc, in_maps, list(range(N_CORES)), trace=trace)
    y = np.concatenate([r["y"] for r in res.results], axis=0)
    return y, res


def kernel(x: np.ndarray, weights: np.ndarray) -> np.ndarray:
    y, _ = _run(x, weights)
    return y
